# revision 32
# baseline (speedup 1.0000x reference)
"""Trainium2 Bass kernel for BinderEnergyGuidance (retrieval_knn).

Math (per batch b of 16):
  d[b,n,m]   = ||binder[b,n] - target[m]||           (N=1024, M=8192)
  attract[b] = mean of the k=204 smallest per-row min-distances
  repel[b]   = sum relu(3 - d)^2
  out[b]     = 10*attract[b] + 5*repel[b]

Strategy: data-parallel over the batch axis, 2 batches per NeuronCore.
Per core:
  - TensorE: d2 = |x|^2 + |y|^2 - 2 x.y as ONE matmul per tile.  The
    K axis uses partition groups at 0/32/64 (compute engines can only
    start at partition multiples of 32; the gap rows are zeroed):
      k 0-2 :  lhsT x_k     . rhs -2*y_k
      k 32-34: lhsT x_k^2   . rhs 1
      k 64-66: lhsT 1       . rhs y_k^2
  - VectorE: tc = clamp(d2, 0, 9) (one tensor_scalar, PSUM->SBUF) and
    per-row min via tensor_reduce(min).  Clamping at 9 is exact:
    clash^2 = (3 - min(d,3))^2 needs no mask, and the 204th-smallest
    min-dist per batch is ~0.7 << 3, so clamped rows never enter top-k.
  - ScalarE: dc = sqrt(tc); Square(3 - dc) with fused per-row
    accumulation -> repel partial sums.
  - Top-k via rank selection: rank_i = #{j : v_j < v_i} with
    tensor_scalar(is_lt)+accumulate against a broadcast row of
    min-dists (PE transpose + DMA flatten + DMA partition-broadcast);
    select rank < 204, dot with v, partition-sum by matmul.

All DMA producers are funneled through V ops where a matmul consumes
them: PE matmul (S3_LW) carries at most 3 semaphore waits and every
DMA completion lands on a different rotating queue semaphore.

Self-contained: hardcodes shapes for binder[16,1024,3], target[8192,3].
"""

import numpy as np
from contextlib import ExitStack

import concourse.bass as bass
import concourse.bacc as bacc
import concourse.tile as tile
from concourse import mybir
from concourse.bass_utils import run_bass_kernel_spmd
from concourse.masks import make_identity

F32 = mybir.dt.float32
AF = mybir.ActivationFunctionType
OP = mybir.AluOpType
AX = mybir.AxisListType

B, N, MT = 16, 1024, 8192
NCORES = 8
BC = B // NCORES          # batches per core
TOPK = 204                # int(0.2 * N)
CLASH = 3.0
CLASH2 = CLASH * CLASH
ATTRACT_SCALE, REPEL_SCALE = 10.0, 5.0

P = 128                   # SBUF partitions
NCHUNK = N // P           # 8 row-chunks per batch
MTILE = 1024              # PSUM tile free size (2 banks)
NMT = MT // MTILE         # m-tiles per row-chunk (8)
MMF = 512                 # fp32 matmul max moving free size
NHALF = 2                 # split M into halves for SBUF working buffers
MHALF = MT // NHALF       # 4096
JPH = NMT // NHALF        # m-tiles per half (4)
KP = 67                   # padded contraction size (groups at 0/32/64)

_prog_cache = {}


def build_program():
    nc = bacc.Bacc("TRN2", target_bir_lowering=False, debug=False,
                   num_devices=NCORES)
    bnd = nc.dram_tensor("bnd", [BC, 3, N], F32, kind="ExternalInput").ap()
    tgt = nc.dram_tensor("tgt", [3, MT], F32, kind="ExternalInput").ap()
    out = nc.dram_tensor("out", [BC, 1], F32, kind="ExternalOutput").ap()

    with tile.TileContext(nc) as tc, ExitStack() as ctx:
        consts = ctx.enter_context(tc.tile_pool(name="consts", bufs=1))
        work = ctx.enter_context(tc.tile_pool(name="work", bufs=1))
        tcp = ctx.enter_context(tc.tile_pool(name="tcp", bufs=2))
        psum = ctx.enter_context(tc.tile_pool(name="psum", bufs=2, space="PSUM"))
        psum2 = ctx.enter_context(tc.tile_pool(name="psum2", bufs=2, space="PSUM"))
        dpool = ctx.enter_context(tc.tile_pool(name="dpool", bufs=1, space="DRAM"))

        # --- build rhs_pad / lhsT_pad with V ops only (DMAs staged) ---
        ys = consts.tile([3, MT], F32)
        nc.sync.dma_start(out=ys[:, :], in_=tgt[:, :])
        rhs_pad = consts.tile([KP, MT], F32)
        nc.vector.memset(rhs_pad[:, :], 0.0)
        nc.vector.tensor_scalar_mul(rhs_pad[0:3, :], ys[:, :], -2.0)
        nc.vector.tensor_mul(rhs_pad[64:67, :], ys[:, :], ys[:, :])
        nc.vector.memset(rhs_pad[32:35, :], 1.0)

        lhsTs = []
        for b in range(BC):
            xs = consts.tile([3, N], F32, name=f"xs{b}")
            nc.sync.dma_start(out=xs[:, :], in_=bnd[b, :, :])
            lhsT_pad = consts.tile([KP, N], F32, name=f"lhsT_pad{b}")
            nc.vector.memset(lhsT_pad[:, :], 0.0)
            nc.vector.tensor_copy(lhsT_pad[0:3, :], xs[:, :])
            nc.vector.tensor_mul(lhsT_pad[32:35, :], xs[:, :], xs[:, :])
            nc.vector.memset(lhsT_pad[64:67, :], 1.0)
            lhsTs.append(lhsT_pad)

        three1 = consts.tile([P, 1], F32)
        nc.vector.memset(three1, CLASH)
        ones128 = consts.tile([P, 1], F32)
        nc.vector.memset(ones128, 1.0)
        ident = consts.tile([P, P], F32)
        make_identity(nc, ident)

        waste_ts = work.tile([P, N], F32)    # rank pass elementwise out (unused)
        dc = work.tile([P, MHALF], F32)      # sqrt(clamped d2)

        for b in range(BC):
            lhsT = lhsTs[b]
            mdB = work.tile([P, NCHUNK], F32, name=f"mdB{b}")      # min d2
            rsB = work.tile([P, NCHUNK * NHALF], F32, name=f"rsB{b}")

            for c in range(NCHUNK):
                lc = lhsT[:, c * P:(c + 1) * P]
                md8 = work.tile([P, NMT], F32, name="md8")
                for h in range(NHALF):
                    tcb = tcp.tile([P, MHALF], F32, name="tcb", tag="tcb")
                    for j4 in range(JPH):
                        j = h * JPH + j4
                        ps = psum.tile([P, MTILE], F32, name="ps", tag="ps")
                        for q in range(MTILE // MMF):
                            nc.tensor.matmul(
                                ps[:, q * MMF:(q + 1) * MMF], lc,
                                rhs_pad[:, j * MTILE + q * MMF:
                                        j * MTILE + (q + 1) * MMF],
                                start=True, stop=True)
                        nc.vector.tensor_scalar(
                            tcb[:, j4 * MTILE:(j4 + 1) * MTILE], ps[:, :],
                            0.0, CLASH2, OP.max, OP.min)
                        nc.vector.tensor_reduce(
                            md8[:, j:j + 1], tcb[:, j4 * MTILE:(j4 + 1) * MTILE],
                            AX.X, OP.min)
                    nc.scalar.activation(dc, tcb, AF.Sqrt)
                    nc.scalar.activation(tcb, dc, AF.Square,
                                         bias=three1[:, 0:1], scale=-1.0,
                                         accum_out=rsB[:, c * NHALF + h:
                                                       c * NHALF + h + 1])
                nc.vector.tensor_reduce(mdB[:, c:c + 1], md8, AX.X, OP.min)

            # ---- per-batch epilogue ----
            vB = work.tile([P, NCHUNK], F32, name=f"vB{b}")   # min dists
            nc.scalar.activation(vB, mdB, AF.Sqrt)

            # vT[c, q] = vB[q, c] (PE transpose), flatten to [1, N], then
            # partition-broadcast to [128, N] -- each step is one DMA so
            # downstream consumers wait on a single producer.
            vT = psum2.tile([NCHUNK, P], F32, name="vT", tag="ep")
            nc.tensor.transpose(vT, vB, ident)
            vTs = work.tile([NCHUNK, P], F32, name=f"vTs{b}")
            nc.scalar.copy(vTs, vT)
            vfl = dpool.tile([1, N], F32, name=f"vfl{b}")
            nc.sync.dma_start(
                out=vfl[0:1, :].rearrange("p (c q) -> p c q", c=NCHUNK),
                in_=vTs[:, :])
            vrep = work.tile([P, N], F32, name=f"vrep{b}")
            vfl_bcast = bass.AP(tensor=vfl.tensor, offset=vfl.offset,
                                ap=[[0, P], vfl.ap[-1]])
            nc.sync.dma_start(out=vrep[:, :], in_=vfl_bcast)

            rank8 = work.tile([P, NCHUNK], F32, name=f"rank8{b}")
            for c in range(NCHUNK):
                nc.vector.tensor_scalar(waste_ts, vrep, vB[:, c:c + 1], 0.0,
                                        OP.is_lt, OP.add,
                                        accum_out=rank8[:, c:c + 1])
            sel8 = work.tile([P, NCHUNK], F32, name=f"sel8{b}")
            nc.vector.tensor_scalar(sel8, rank8, float(TOPK), None, OP.is_lt)

            stack2 = work.tile([P, 2], F32, name=f"stack2{b}")
            prod8 = work.tile([P, NCHUNK], F32, name=f"prod8{b}")
            nc.vector.tensor_mul(prod8, sel8, vB)
            nc.vector.tensor_reduce(stack2[:, 0:1], prod8, AX.X, OP.add)
            nc.vector.tensor_reduce(stack2[:, 1:2], rsB, AX.X, OP.add)

            fin = psum2.tile([1, 2], F32, name="fin", tag="ep")
            nc.tensor.matmul(fin, ones128, stack2, start=True, stop=True)
            en = work.tile([1, 2], F32, name=f"en{b}")
            nc.vector.tensor_scalar_mul(en[0:1, 0:1], fin[0:1, 0:1],
                                        ATTRACT_SCALE / TOPK)
            nc.vector.tensor_scalar_mul(en[0:1, 1:2], fin[0:1, 1:2],
                                        REPEL_SCALE)
            en2 = work.tile([1, 1], F32, name=f"en2{b}")
            nc.vector.tensor_add(en2, en[0:1, 0:1], en[0:1, 1:2])
            nc.sync.dma_start(out=out[b:b + 1, 0:1], in_=en2[0:1, 0:1])

    nc.compile()
    return nc


def _get_program():
    if "nc" not in _prog_cache:
        _prog_cache["nc"] = build_program()
    return _prog_cache["nc"]


def make_in_maps(binder_trans, target_coords):
    x = np.ascontiguousarray(
        np.asarray(binder_trans, dtype=np.float32).transpose(0, 2, 1))
    y = np.ascontiguousarray(np.asarray(target_coords, dtype=np.float32).T)
    return [{"bnd": np.ascontiguousarray(x[c * BC:(c + 1) * BC]), "tgt": y}
            for c in range(NCORES)]


def kernel(binder_trans, target_coords):
    nc = _get_program()
    in_maps = make_in_maps(binder_trans, target_coords)
    res = run_bass_kernel_spmd(nc, in_maps, list(range(NCORES)))
    outs = [np.asarray(res.results[c]["out"], dtype=np.float32).reshape(BC)
            for c in range(NCORES)]
    return np.concatenate(outs).astype(np.float32)


# revision 42
# speedup vs baseline: 597.3865x; 597.3865x over previous
"""Trainium2 Bass kernel for BinderEnergyGuidance (retrieval_knn).

Math (per batch b of 16):
  d[b,n,m]   = ||binder[b,n] - target[m]||           (N=1024, M=8192)
  attract[b] = mean of the k=204 smallest per-row min-distances
  repel[b]   = sum relu(3 - d)^2
  out[b]     = 10*attract[b] + 5*repel[b]

Strategy: data-parallel over the batch axis, 2 batches per NeuronCore.
Per core:
  - TensorE: d2 = |x|^2 + |y|^2 - 2 x.y as ONE matmul per tile.  The
    K axis uses partition groups at 0/32/64 (compute engines can only
    start at partition multiples of 32; the gap rows are zeroed):
      k 0-2 :  lhsT x_k     . rhs -2*y_k
      k 32-34: lhsT x_k^2   . rhs 1
      k 64-66: lhsT 1       . rhs y_k^2
  - VectorE: tc = clamp(d2, 0, 9) (one tensor_scalar, PSUM->SBUF) and
    per-row min via tensor_reduce(min).  Clamping at 9 is exact:
    clash^2 = (3 - min(d,3))^2 needs no mask, and the 204th-smallest
    min-dist per batch is ~0.7 << 3, so clamped rows never enter top-k.
  - ScalarE: dc = sqrt(tc); Square(3 - dc) with fused per-row
    accumulation -> repel partial sums.
  - Top-k via rank selection: rank_i = #{j : v_j < v_i} with
    tensor_scalar(is_lt)+accumulate against a broadcast row of
    min-dists (PE transpose + DMA flatten + DMA partition-broadcast);
    select rank < 204, dot with v, partition-sum by matmul.

All DMA producers are funneled through V ops where a matmul consumes
them: PE matmul (S3_LW) carries at most 3 semaphore waits and every
DMA completion lands on a different rotating queue semaphore.

Self-contained: hardcodes shapes for binder[16,1024,3], target[8192,3].
"""

import numpy as np
from contextlib import ExitStack

import concourse.bass as bass
import concourse.bacc as bacc
import concourse.tile as tile
from concourse import mybir
from concourse.bass_utils import run_bass_kernel_spmd
from concourse.masks import make_identity

F32 = mybir.dt.float32
F32R = mybir.dt.float32r
AF = mybir.ActivationFunctionType
OP = mybir.AluOpType
AX = mybir.AxisListType

B, N, MT = 16, 1024, 8192
NCORES = 8
BC = B // NCORES          # batches per core
TOPK = 204                # int(0.2 * N)
CLASH = 3.0
CLASH2 = CLASH * CLASH
ATTRACT_SCALE, REPEL_SCALE = 10.0, 5.0

P = 128                   # SBUF partitions
NCHUNK = N // P           # 8 row-chunks per batch
MTILE = 1024              # PSUM tile free size (2 banks)
NMT = MT // MTILE         # m-tiles per row-chunk (8)
MMF = 512                 # fp32 matmul max moving free size
NHALF = 2                 # split M into halves for SBUF working buffers
MHALF = MT // NHALF       # 4096
JPH = NMT // NHALF        # m-tiles per half (4)
KP = 67                   # padded contraction size (groups at 0/32/64)

_prog_cache = {}


def build_program():
    nc = bacc.Bacc("TRN2", target_bir_lowering=False, debug=False,
                   num_devices=NCORES)
    bnd = nc.dram_tensor("bnd", [BC, 3, N], F32, kind="ExternalInput").ap()
    tgt = nc.dram_tensor("tgt", [3, MT], F32, kind="ExternalInput").ap()
    out = nc.dram_tensor("out", [BC, 1], F32, kind="ExternalOutput").ap()

    with tile.TileContext(nc) as tc, ExitStack() as ctx:
        consts = ctx.enter_context(tc.tile_pool(name="consts", bufs=1))
        work = ctx.enter_context(tc.tile_pool(name="work", bufs=1))
        tcp = ctx.enter_context(tc.tile_pool(name="tcp", bufs=2))
        psum = ctx.enter_context(tc.tile_pool(name="psum", bufs=3, space="PSUM"))
        psum2 = ctx.enter_context(tc.tile_pool(name="psum2", bufs=2, space="PSUM"))
        dpool = ctx.enter_context(tc.tile_pool(name="dpool", bufs=1, space="DRAM"))

        # --- build rhs_pad / lhsT_pad with V ops only (DMAs staged).
        # fp32r matmul operands must come from fp32r-rounding producers,
        # so zero/one fills go through copies/tensor_scalar, not memset. ---
        ys = consts.tile([3, MT], F32)
        nc.sync.dma_start(out=ys[:, :], in_=tgt[:, :])
        rhs_pad = consts.tile([KP, MT], F32R)
        lhsTs = [consts.tile([KP, N], F32R, name=f"lhsT_pad{b}")
                 for b in range(BC)]
        with tc.tile_pool(name="zscr", bufs=1) as zscr:
            zKP = zscr.tile([KP, MT], F32)
            nc.vector.memset(zKP[:, :], 0.0)
            nc.vector.tensor_copy(rhs_pad[:, :], zKP[:, :])
            nc.vector.tensor_scalar_mul(rhs_pad[0:3, :], ys[:, :], -2.0)
            nc.vector.tensor_mul(rhs_pad[64:67, :], ys[:, :], ys[:, :])
            nc.vector.tensor_scalar(rhs_pad[32:35, :], ys[:, :], 0.0, 1.0,
                                    OP.mult, OP.add)
            for b in range(BC):
                xs = consts.tile([3, N], F32, name=f"xs{b}")
                nc.sync.dma_start(out=xs[:, :], in_=bnd[b, :, :])
                lhsT_pad = lhsTs[b]
                nc.vector.tensor_copy(lhsT_pad[:, :], zKP[:, 0:N])
                nc.vector.tensor_copy(lhsT_pad[0:3, :], xs[:, :])
                nc.vector.tensor_mul(lhsT_pad[32:35, :], xs[:, :], xs[:, :])
                nc.vector.tensor_scalar(lhsT_pad[64:67, :], xs[:, :], 0.0, 1.0,
                                        OP.mult, OP.add)

        three1 = consts.tile([P, 1], F32)
        nc.vector.memset(three1, CLASH)
        ones128 = consts.tile([P, 1], F32)
        nc.vector.memset(ones128, 1.0)
        ident = consts.tile([P, P], F32)
        make_identity(nc, ident)

        waste_ts = work.tile([P, N], F32)    # rank pass elementwise out (unused)

        for b in range(BC):
            lhsT = lhsTs[b]
            mdB = work.tile([P, NCHUNK], F32, name=f"mdB{b}")      # min d2
            dcsB = work.tile([P, NCHUNK * NHALF], F32, name=f"dcsB{b}")
            stcB = work.tile([P, NCHUNK], F32, name=f"stcB{b}")

            for c in range(NCHUNK):
                lc = lhsT[:, c * P:(c + 1) * P]
                md8 = work.tile([P, NMT], F32, name="md8")
                stc8 = work.tile([P, NMT], F32, name="stc8")
                for h in range(NHALF):
                    tcl = tcp.tile([P, MHALF], F32, name="tcl", tag="tcl")
                    tcb = tcp.tile([P, MHALF], F32, name="tcb", tag="tcb")
                    for j4 in range(JPH):
                        j = h * JPH + j4
                        sl = slice(j4 * MTILE, (j4 + 1) * MTILE)
                        ps = psum.tile([P, MTILE], F32, name="ps", tag="ps")
                        for q in range(MTILE // MMF):
                            nc.tensor.matmul(
                                ps[:, q * MMF:(q + 1) * MMF], lc,
                                rhs_pad[:, j * MTILE + q * MMF:
                                        j * MTILE + (q + 1) * MMF],
                                start=True, stop=True)
                        # pass 1: lower clamp + fused true-min accumulate
                        nc.vector.tensor_scalar(
                            tcl[:, sl], ps[:, :], 0.0, 3.4e38,
                            OP.max, OP.min, accum_out=md8[:, j:j + 1])
                        # pass 2: upper clamp + fused sum accumulate (2x mode)
                        nc.vector.tensor_scalar(
                            tcb[:, sl], tcl[:, sl], CLASH2, 0.0,
                            OP.min, OP.add, accum_out=stc8[:, j:j + 1])
                    # sqrt of clamped d2; only the row-sum accumulator is used:
                    # repel_row = 9*M - 6*sum(dc) + sum(tc)
                    nc.scalar.activation(tcl, tcb, AF.Sqrt,
                                         accum_out=dcsB[:, c * NHALF + h:
                                                        c * NHALF + h + 1])
                nc.vector.tensor_reduce(mdB[:, c:c + 1], md8, AX.X, OP.min)
                nc.vector.tensor_reduce(stcB[:, c:c + 1], stc8, AX.X, OP.add)

            # ---- per-batch epilogue ----
            vB = work.tile([P, NCHUNK], F32, name=f"vB{b}")   # min dists
            nc.scalar.activation(vB, mdB, AF.Sqrt)

            # vT[c, q] = vB[q, c] (PE transpose), flatten to [1, N], then
            # partition-broadcast to [128, N] -- each step is one DMA so
            # downstream consumers wait on a single producer.
            vT = psum2.tile([NCHUNK, P], F32, name="vT", tag="ep")
            nc.tensor.transpose(vT, vB, ident)
            vTs = work.tile([NCHUNK, P], F32, name=f"vTs{b}")
            nc.scalar.copy(vTs, vT)
            vfl = dpool.tile([1, N], F32, name=f"vfl{b}")
            nc.sync.dma_start(
                out=vfl[0:1, :].rearrange("p (c q) -> p c q", c=NCHUNK),
                in_=vTs[:, :])
            vrep = work.tile([P, N], F32, name=f"vrep{b}")
            vfl_bcast = bass.AP(tensor=vfl.tensor, offset=vfl.offset,
                                ap=[[0, P], vfl.ap[-1]])
            nc.sync.dma_start(out=vrep[:, :], in_=vfl_bcast)

            rank8 = work.tile([P, NCHUNK], F32, name=f"rank8{b}")
            for c in range(NCHUNK):
                nc.vector.tensor_scalar(waste_ts, vrep, vB[:, c:c + 1], 0.0,
                                        OP.is_lt, OP.add,
                                        accum_out=rank8[:, c:c + 1])
            sel8 = work.tile([P, NCHUNK], F32, name=f"sel8{b}")
            nc.vector.tensor_scalar(sel8, rank8, float(TOPK), None, OP.is_lt)

            stack2 = work.tile([P, 2], F32, name=f"stack2{b}")
            prod8 = work.tile([P, NCHUNK], F32, name=f"prod8{b}")
            nc.vector.tensor_mul(prod8, sel8, vB)
            nc.vector.tensor_reduce(stack2[:, 0:1], prod8, AX.X, OP.add)
            # per-row repel: 9*M - 6*sum(dc) + sum(tc)
            tdc = work.tile([P, 1], F32, name=f"tdc{b}")
            nc.vector.tensor_reduce(tdc, dcsB, AX.X, OP.add)
            tst = work.tile([P, 1], F32, name=f"tst{b}")
            nc.vector.tensor_reduce(tst, stcB, AX.X, OP.add)
            tdc2 = work.tile([P, 1], F32, name=f"tdc2{b}")
            nc.vector.tensor_scalar(tdc2, tdc, -6.0, float(9 * MT * NCHUNK),
                                    OP.mult, OP.add)
            nc.vector.tensor_add(stack2[:, 1:2], tdc2, tst)

            fin = psum2.tile([1, 2], F32, name="fin", tag="ep")
            nc.tensor.matmul(fin, ones128, stack2, start=True, stop=True)
            en = work.tile([1, 2], F32, name=f"en{b}")
            nc.vector.tensor_scalar_mul(en[0:1, 0:1], fin[0:1, 0:1],
                                        ATTRACT_SCALE / TOPK)
            nc.vector.tensor_scalar_mul(en[0:1, 1:2], fin[0:1, 1:2],
                                        REPEL_SCALE)
            en2 = work.tile([1, 1], F32, name=f"en2{b}")
            nc.vector.tensor_add(en2, en[0:1, 0:1], en[0:1, 1:2])
            nc.sync.dma_start(out=out[b:b + 1, 0:1], in_=en2[0:1, 0:1])

    nc.compile()
    return nc


def _get_program():
    if "nc" not in _prog_cache:
        _prog_cache["nc"] = build_program()
    return _prog_cache["nc"]


def make_in_maps(binder_trans, target_coords):
    x = np.ascontiguousarray(
        np.asarray(binder_trans, dtype=np.float32).transpose(0, 2, 1))
    y = np.ascontiguousarray(np.asarray(target_coords, dtype=np.float32).T)
    return [{"bnd": np.ascontiguousarray(x[c * BC:(c + 1) * BC]), "tgt": y}
            for c in range(NCORES)]


def kernel(binder_trans, target_coords):
    nc = _get_program()
    in_maps = make_in_maps(binder_trans, target_coords)
    res = run_bass_kernel_spmd(nc, in_maps, list(range(NCORES)))
    outs = [np.asarray(res.results[c]["out"], dtype=np.float32).reshape(BC)
            for c in range(NCORES)]
    return np.concatenate(outs).astype(np.float32)
